# revision 22
# baseline (speedup 1.0000x reference)
"""Cross-attention + parallel-FF block on 8 Trainium2 cores (Bass/Tile).

Sharding: rows of x (sequence-parallel). Each core processes 512 of the 4096
query rows: LN, q-projection (all 8 heads), full attention over the shared
(multi-query) K/V, output projection and the full FF for its rows. K/V are
small so every core computes them from the full context (replicated work, no
collectives). The output is a row-concat of per-core results.

Structure:
- All feature transposes (xn^T, ctx^T, kv row-major) run on the DMA engines
  (InstDmaTransposeAnt): zero PE/PSUM cost. HW constraint: output must be
  contiguous 128-col blocks (strided 66-pitch writes corrupt), so kv is
  transposed full-width and v carved out with a lane-wise strided copy.
- k/v projection fused into one [128kv x 128j] matmul per context tile; k^T
  falls out directly.
- Attention runs in fp8(e4m3) DoubleRow: sim contracts dh=64 as 2x32
  partition tiles, attn@v contracts j as 2x128 tiles, and the output
  projection contracts inner=512 as 2x128 pairs — each at 0.5 PE
  cycles/row. fp8 quantization washes out in the softmax average; the FF
  path (which dominates max|out|) stays bf16.
- ff1 tiles interleaved into both the context phase and the attention phase
  to keep the PE saturated while Act does LN/exp.

Numerics: layer norm stats in fp32; FF/projection matmuls in bf16 with fp32
PSUM accumulation. gamma / softmax scale are folded into the weights on the
host. Softmax skips the amax subtraction (sim values are O(1) by
construction); the denominator comes from a ones-column appended to V.
"""

import numpy as np
import ml_dtypes

import concourse.bass as bass
import concourse.tile as tile
from concourse import bacc, mybir
from concourse.bass import ts
from concourse.masks import make_identity

BF16 = mybir.dt.bfloat16
F32 = mybir.dt.float32
FP8 = mybir.dt.float8e4
DR = mybir.MatmulPerfMode.DoubleRow

N_CORES = 8
N = 4096            # query rows (total)
NS = N // N_CORES   # rows per core = 512
D = 1024            # model dim
J = 4096            # context rows
H = 8               # heads
DH = 64             # head dim
INNER = H * DH      # 512
FF = 4096           # ff_inner
EPS = 1e-5

NT = NS // 128      # 4   query-row tiles per core
JT = J // 128       # 32  context-row tiles
DC = D // 128       # 8   feature chunks
FT = FF // 128      # 32  ff tiles (per a/gate half)

FF_P3 = 9           # ff1 tiles emitted during the context phase


def build(reps=1, ff_p3=FF_P3, phases=(1, 2, 3, 4, 6)):
    nc = bacc.Bacc("TRN2", target_bir_lowering=False, debug=False,
                   num_devices=N_CORES)

    xs_d = nc.dram_tensor("xs", [NS, D], BF16, kind="ExternalInput")
    ctx_d = nc.dram_tensor("ctx", [J, D], BF16, kind="ExternalInput")
    wq_d = nc.dram_tensor("wq", [D, INNER], BF16, kind="ExternalInput")
    wkv_d = nc.dram_tensor("wkv", [D, 2 * DH], BF16, kind="ExternalInput")
    wout_d = nc.dram_tensor("wout", [INNER, D], BF16, kind="ExternalInput")
    wff1_d = nc.dram_tensor("wff1", [2 * FT, 128, DC, 128], BF16, kind="ExternalInput")
    wff2_d = nc.dram_tensor("wff2", [FF, D], BF16, kind="ExternalInput")
    out_d = nc.dram_tensor("out", [NS, D], F32, kind="ExternalOutput")

    with tile.TileContext(nc) as tc:
        with (
            tc.tile_pool(name="const", bufs=1) as constp,
            tc.tile_pool(name="weights", bufs=1) as wp,
            tc.tile_pool(name="resident", bufs=1) as rp,
            tc.tile_pool(name="ln1", bufs=1) as ln1,
            tc.tile_pool(name="ctload", bufs=2) as ctl,
            tc.tile_pool(name="work", bufs=2) as work,
            tc.tile_pool(name="expt", bufs=5) as expp,
            tc.tile_pool(name="small", bufs=8) as small,
            tc.tile_pool(name="wstream", bufs=5) as ws,
        ):
            ident = constp.tile([128, 128], BF16)
            make_identity(nc, ident[:])
            eps_t = constp.tile([128, 1], F32)
            nc.gpsimd.memset(eps_t[:], EPS)

            wq_sb = wp.tile([128, DC, INNER], BF16)
            nc.sync.dma_start(wq_sb[:], wq_d.ap().rearrange("(c p) n -> p c n", p=128))
            wkv_sb = wp.tile([128, DC, 2 * DH], BF16)
            nc.sync.dma_start(wkv_sb[:], wkv_d.ap().rearrange("(c p) n -> p c n", p=128))
            wout_sb = wp.tile([128, INNER // 128, D], BF16)
            nc.sync.dma_start(wout_sb[:], wout_d.ap().rearrange("(c p) n -> p c n", p=128))

            xnT = rp.tile([128, DC, NS], BF16)       # LN(x)^T   [f, i]
            qT8 = rp.tile([32, 2, H, NS], FP8)       # q^T fp8, dh split 2x32
            q8hi = rp.tile([64, H, NS], FP8)         # staging: dh 32:64 lanes
            kvT = rp.tile([128, JT, 128], BF16)      # [k^T | v^T] per ctx tile
            kT8 = rp.tile([32, 2, JT, 128], FP8)     # k^T fp8, dh split 2x32
            kv8hi = rp.tile([64, JT, 128], FP8)      # staging: dh 32:64 lanes
            kvrm = rp.tile([128, JT, 128], BF16)     # kv row-major [j, k|v]
            v8 = rp.tile([128, JT // 2, 2, DH + 2], FP8)  # v + ones col, fp8
            oT = rp.tile([128, INNER // 128, NS], BF16)   # attn-out^T [inner, i]
            pT = rp.tile([128, FT, NS], BF16)        # (a*gate)^T [ff, i]

            nc.gpsimd.memset(v8[:, :, :, DH:DH + 1], 1.0)

            def ln_norm(x_ap, xn_ap):
                """bn_stats layer norm: xn = (x - mu) * rsqrt(var + eps)."""
                stats = small.tile([128, 2, 6], F32, tag="stats")
                mv = small.tile([128, 2], F32, tag="mv")
                sq = small.tile([128, 1], F32, tag="sq")
                r = small.tile([128, 1], F32, tag="r")
                nmr = small.tile([128, 1], F32, tag="nmr")
                xv = x_ap.rearrange("p (a b) -> p a b", b=512)
                nc.vector.bn_stats(stats[:, 0, :], xv[:, 0, :])
                nc.vector.bn_stats(stats[:, 1, :], xv[:, 1, :])
                nc.vector.bn_aggr(mv[:], stats[:])
                nc.scalar.activation(sq[:], mv[:, 1:2],
                                     mybir.ActivationFunctionType.Sqrt,
                                     bias=eps_t[:], scale=1.0)
                nc.vector.reciprocal(r[:], sq[:])
                nc.vector.scalar_tensor_tensor(nmr[:], mv[:, 0:1], -1.0, r[:],
                                               op0=mybir.AluOpType.mult,
                                               op1=mybir.AluOpType.mult)
                nc.scalar.activation(xn_ap, x_ap,
                                     mybir.ActivationFunctionType.Identity,
                                     bias=nmr[:], scale=r[:])

            def ff1_tile(t, ps_ff):
                wa = ws.tile([128, DC, 128], BF16, tag="wa")
                nc.sync.dma_start(wa[:], wff1_d.ap()[t])
                wg = ws.tile([128, DC, 128], BF16, tag="wg")
                nc.sync.dma_start(wg[:], wff1_d.ap()[t + FT])
                ha = ps_ff.tile([128, NS], F32, tag="ha")
                hg = ps_ff.tile([128, NS], F32, tag="hg")
                for c in range(DC):
                    nc.tensor.matmul(ha[:], wa[:, c, :], xnT[:, c, :],
                                     start=(c == 0), stop=(c == DC - 1))
                for c in range(DC):
                    nc.tensor.matmul(hg[:], wg[:, c, :], xnT[:, c, :],
                                     start=(c == 0), stop=(c == DC - 1))
                ha_sb = ws.tile([128, NS], BF16, tag="ha_sb")
                nc.vector.tensor_copy(ha_sb[:], ha[:])
                nc.vector.tensor_mul(pT[:, t, :], ha_sb[:], hg[:])

            def body():
                with tc.tile_pool(name="ps_ff", bufs=1,
                                  space=bass.MemorySpace.PSUM) as ps_ff:
                    ff_next = [0]

                    with (
                        tc.tile_pool(name="ps_q", bufs=2,
                                     space=bass.MemorySpace.PSUM) as ps_q,
                        tc.tile_pool(name="ps_kv", bufs=2,
                                     space=bass.MemorySpace.PSUM) as ps_kv,
                    ):
                        # ---- phase 1: LN(x shard), DMA-transpose -> xnT ----
                        xt4 = ln1.tile([128, NT, D], BF16, tag="xt4")
                        nc.sync.dma_start(
                            xt4[:], xs_d.ap().rearrange("(a p) d -> p a d", p=128))

                        def load_ct4(u):
                            t = ctl.tile([128, 4, D], BF16, tag="ct4")
                            nc.sync.dma_start(
                                t[:],
                                ctx_d.ap()[512 * u:512 * (u + 1), :]
                                .rearrange("(a p) d -> p a d", p=128))
                            return t

                        ct4s = {0: load_ct4(0)}

                        xn4 = ln1.tile([128, NT, D], BF16, tag="xn4")
                        for it in range(NT):
                            ln_norm(xt4[:, it, :], xn4[:, it, :])
                            nc.sync.dma_start_transpose(
                                xnT[:, :, ts(it, 128)], xn4[:, it, :])

                        # ---- phase 2: q projection -> qT8 (fp8, dh-split) ----
                        for h in range(H):
                            qp = ps_q.tile([64, NS], F32, tag="qp")
                            for c in range(DC):
                                nc.tensor.matmul(qp[:], wq_sb[:, c, ts(h, 64)],
                                                 xnT[:, c, :],
                                                 start=(c == 0), stop=(c == DC - 1))
                            nc.vector.tensor_copy(qT8[:, 0, h, :], qp[0:32, :])
                            nc.vector.tensor_copy(q8hi[32:64, h, :], qp[32:64, :])
                        nc.sync.dma_start(qT8[:, 1, :, :], q8hi[32:64, :, :])

                        # early ff1 tiles fill the PE while ctx LN warms up
                        for _ in range(2):
                            if ff_next[0] < ff_p3:
                                ff1_tile(ff_next[0], ps_ff)
                                ff_next[0] += 1

                        # ---- phase 3: LN(ctx), DMA-transpose, fused kv ----
                        for u in range(JT // 4 if 3 in phases else 0):
                            ct4 = ct4s.pop(u)
                            if u + 1 < JT // 4:
                                ct4s[u + 1] = load_ct4(u + 1)
                            cn4 = work.tile([128, 4, D], BF16, tag="cn4")
                            for pr in range(2):
                                for a in (2 * pr, 2 * pr + 1):
                                    ln_norm(ct4[:, a, :], cn4[:, a, :])
                                cT2 = work.tile([128, 2 * DC, 128], BF16, tag="cT2")
                                nc.sync.dma_start_transpose(
                                    cT2[:], cn4[:, 2 * pr:2 * pr + 2, :])
                                for s in (0, 1):
                                    jt = 4 * u + 2 * pr + s
                                    kvp = ps_kv.tile([128, 128], F32, tag="kvp")
                                    for c in range(DC):
                                        nc.tensor.matmul(kvp[:], wkv_sb[:, c, :],
                                                         cT2[:, DC * s + c, :],
                                                         start=(c == 0),
                                                         stop=(c == DC - 1))
                                    nc.vector.tensor_copy(kvT[:, jt, :], kvp[:])
                                if pr == 1 and u % 2 == 0 and ff_next[0] < ff_p3:
                                    ff1_tile(ff_next[0], ps_ff)
                                    ff_next[0] += 1
                            # fp8 k^T for this u's 4 tiles (dh-split lanes)
                            nc.vector.tensor_copy(kT8[:, 0, ts(u, 4), :],
                                                  kvT[0:32, ts(u, 4), :])
                            nc.vector.tensor_copy(kv8hi[32:64, ts(u, 4), :],
                                                  kvT[32:64, ts(u, 4), :])
                        while ff_next[0] < ff_p3:
                            ff1_tile(ff_next[0], ps_ff)
                            ff_next[0] += 1

                        if 3 not in phases:
                            ct4s.pop(0)
                        nc.sync.dma_start(kT8[:, 1, :, :], kv8hi[32:64, :, :])
                        # kv row-major via full-width DMA transpose; v8 = fp8
                        # strided carve-out of the v half (+ ones col kept).
                        nc.sync.dma_start_transpose(kvrm[:], kvT[:])
                        v8v = v8[:].rearrange("p a b c -> p (a b) c")
                        nc.vector.tensor_copy(v8v[:, :, 0:DH], kvrm[:, :, DH:128])

                    # ---- phase 4: attention per head (ff1 interleaved) ----
                    with (
                        tc.tile_pool(name="ps_sim", bufs=2,
                                     space=bass.MemorySpace.PSUM) as ps_sim,
                        tc.tile_pool(name="ps_ao", bufs=1,
                                     space=bass.MemorySpace.PSUM) as ps_ao,
                        tc.tile_pool(name="ps_ot", bufs=1,
                                     space=bass.MemorySpace.PSUM) as ps_ot,
                    ):
                        n_ff4 = FT - ff_p3
                        for h in range(H if 4 in phases else 0):
                            ao = ps_ao.tile([128, NT, DH + 2], F32, tag="ao")
                            for jp in range(JT // 2):
                                sim = ps_sim.tile([128, 2, NS], F32, tag="sim")
                                for u in range(2):
                                    jt = 2 * jp + u
                                    nc.tensor.matmul(sim[:, u, :],
                                                     kT8[:, :, jt, :],
                                                     qT8[:, :, h, :],
                                                     start=True, stop=True,
                                                     perf_mode=DR)
                                et = expp.tile([128, 2, NS], FP8, tag="et")
                                nc.scalar.activation(et[:], sim[:],
                                                     mybir.ActivationFunctionType.Exp,
                                                     bias=0.0, scale=1.0 / 64.0)
                                for ib in range(NT):
                                    nc.tensor.matmul(
                                        ao[:, ib, 0:DH + 1],
                                        et[:, :, ts(ib, 128)],
                                        v8[:, jp, :, 0:DH + 1],
                                        start=(jp == 0 and ib == 0),
                                        stop=(jp == JT // 2 - 1 and ib == NT - 1),
                                        perf_mode=DR)
                            for ib in range(NT):
                                rec = small.tile([128, 1], F32, tag="rec")
                                nc.vector.reciprocal(rec[:], ao[:, ib, DH:DH + 1])
                                ob = small.tile([128, DH], BF16, tag="ob")
                                nc.vector.tensor_scalar_mul(ob[:], ao[:, ib, 0:DH],
                                                            rec[:])
                                otp = ps_ot.tile([64, 128], BF16, tag="otp")
                                nc.tensor.transpose(otp[:], ob[:], ident[:])
                                nc.vector.tensor_copy(
                                    oT[64 * (h % 2):64 * (h % 2) + 64, h // 2,
                                       ts(ib, 128)],
                                    otp[:])
                            t0 = ff_p3 + (h * n_ff4) // H
                            t1 = ff_p3 + ((h + 1) * n_ff4) // H
                            for t in range(t0, t1):
                                ff1_tile(t, ps_ff)

                # ---- phase 6: out = pT^T @ Wff2 + oT^T @ Wout ----
                # ff2 accumulates first; the fp8 DoubleRow out-projection
                # finishes each PSUM tile separately so copy/store pipeline
                # with the remaining tiles' matmuls.
                if 6 not in phases:
                    return
                with tc.tile_pool(name="ps_out", bufs=1,
                                  space=bass.MemorySpace.PSUM) as ps_out:
                    op = [[None] * 2 for _ in range(NT)]
                    for ib in range(NT):
                        for fh in range(2):
                            op_t = ps_out.tile([128, 512], F32, tag=f"op{ib}{fh}")
                            op[ib][fh] = op_t
                    for t in range(FT):
                        w2 = ws.tile([128, D], BF16, tag="w2")
                        nc.sync.dma_start(w2[:], wff2_d.ap()[ts(t, 128), :])
                        for ib in range(NT):
                            for fh in range(2):
                                nc.tensor.matmul(op[ib][fh][:],
                                                 pT[:, t, ts(ib, 128)],
                                                 w2[:, ts(fh, 512)],
                                                 start=(t == 0), stop=False)
                    for ib in range(NT):
                        for fh in range(2):
                            for c in range(INNER // 128):
                                nc.tensor.matmul(op[ib][fh][:],
                                                 oT[:, c, ts(ib, 128)],
                                                 wout_sb[:, c, ts(fh, 512)],
                                                 start=False,
                                                 stop=(c == INNER // 128 - 1))
                            ob_sb = work.tile([128, 512], F32, tag="ob_sb")
                            nc.scalar.copy(ob_sb[:], op[ib][fh][:])
                            nc.sync.dma_start(out_d.ap()[ts(ib, 128), ts(fh, 512)],
                                              ob_sb[:])

            if reps == 1:
                body()
            elif reps < 0:  # negative: python-unrolled (sim experiments)
                for _ in range(-reps):
                    body()
            else:
                with tc.For_i(0, reps, 1):
                    body()
    nc.compile()
    return nc


_CACHE = {}


def _get_nc(reps=1):
    if reps not in _CACHE:
        _CACHE[reps] = build(reps)
    return _CACHE[reps]


def _prep_inputs(x, context, gamma, ctx_gamma, Wq, Wkv, Wout, Wff1, Wff2):
    bf = ml_dtypes.bfloat16
    f8 = ml_dtypes.float8_e4m3
    gamma = np.asarray(gamma, np.float32)
    ctx_gamma = np.asarray(ctx_gamma, np.float32)
    scale = 1.0 / np.sqrt(DH)
    wq = (gamma[:, None] * np.asarray(Wq, np.float32) * scale * 8.0).astype(bf)
    kv_gain = np.concatenate([np.full(DH, 8.0, np.float32), np.ones(DH, np.float32)])
    wkv = (ctx_gamma[:, None] * np.asarray(Wkv, np.float32) * kv_gain).astype(bf)
    wout = np.asarray(Wout, np.float32).astype(bf)
    wff1 = (gamma[:, None] * np.asarray(Wff1, np.float32)).astype(bf)
    wff1 = wff1.reshape(DC, 128, 2 * FT, 128).transpose(2, 1, 0, 3).copy()
    wff2 = np.asarray(Wff2, np.float32).astype(bf)
    x = np.asarray(x, np.float32)
    context = np.asarray(context, np.float32)
    in_maps = []
    for c in range(N_CORES):
        in_maps.append({
            "xs": np.ascontiguousarray(x[c * NS:(c + 1) * NS]).astype(bf),
            "ctx": context.astype(bf),
            "wq": wq, "wkv": wkv, "wout": wout, "wff1": wff1, "wff2": wff2,
        })
    return in_maps


def kernel(x, context, gamma, ctx_gamma, Wq, Wkv, Wout, Wff1, Wff2, batch=None,
           **_unused):
    from concourse.bass_utils import run_bass_kernel_spmd

    nc = _get_nc(1)
    in_maps = _prep_inputs(x, context, gamma, ctx_gamma, Wq, Wkv, Wout, Wff1, Wff2)
    res = run_bass_kernel_spmd(nc, in_maps, list(range(N_CORES)))
    return np.concatenate([res.results[c]["out"] for c in range(N_CORES)], axis=0)
